# revision 28
# baseline (speedup 1.0000x reference)
"""Trainium2 Bass kernel for an autoregressive-flow (MAF) layer.

Reference computation (per region r, batch-network b):
    xr[n, d]   = x[n, region_idx[r, d]]                      # [N, D]
    h1 = relu(xr @ (W1*M1)[r,b])                             # [N, H]
    h2 = relu(h1 @ (W2*M2)[r,b])                             # [N, H]
    o  = h2 @ (W3*M3)[r,b]                                   # [N, 2D]
    shift = o[:, 0::2]; log_scale = o[:, 1::2]
    u  = (xr - shift) * exp(-log_scale)
    ll[n, r, b] = sum_d(-0.5*u^2 - 0.5*log(2*pi) - log_scale)

Sharding: region axis R=8 across the 8 NeuronCores; each core handles its
region's B=16 networks over all N=2048 samples.

v2 design notes (what changed vs the original baseline and why):
  - The kernel is PSUM-evacuation-bound, not matmul-bound: GpSimd cannot
    touch PSUM on TRN2, so the 32 per-chunk relus (PSUM fp32 -> SBUF bf16)
    plus the tail's PSUM reads all land on ACT+DVE.  Everything below
    attacks that wall:
      * L1/L2 matmul pairs write one [128, 2, 512] two-bank PSUM tile, so
        each relu covers TWO networks in one [128, 1024]-free op (engine
        cost is free-size based; this halves the per-op overhead count).
      * The tail is restructured to E = 0.5*u^2 + log_scale with the 0.5
        folded into the Exp bias (exp(-2l + ln 0.5)), so ONE block-ones
        matmul per group does the whole ll reduction and PSUM is read only
        three times per group (sub, exp, scalar_tensor_tensor).
      * The SBUF-only tail multiplies (T*T, A*B) go to GpSimd.
      * The -D*0.5*log(2pi) bias is added on the host during the gather.
  - Matmuls stay bf16 and use tile_position packing for hardware
    concurrency: L1 row-packs 4 K=32 matmuls (measured ~3x), L3 col-packs
    4 M=32 matmuls per PSUM tile.  L2 is 16 full-array matmuls with
    automatic fast-weight-load.  (fp8 DoubleRow was tried and is a net
    loss here: it serializes the packs, pays +72% LDWEIGHTS, and the PE
    is not the bottleneck.)
  - Stages are software-pipelined across chunks (L1(c) | L2(c-1) | X(c-2))
    so the PE queue never sits behind an unmet dependency and the HAM
    clock-gate stays warm.
"""

import ml_dtypes
import numpy as np

import concourse.bacc as bacc
import concourse.mybir as mybir
from concourse.bass_utils import run_bass_kernel_spmd
from concourse.tile import TileContext

R, B, D, H, N, F = 8, 16, 32, 128, 2048, 256
HALF_LOG_2PI = 0.9189385332046727
N_CORES = 8
CHUNK = 512
F32 = mybir.dt.float32
BF16 = mybir.dt.bfloat16
LN_HALF = -0.6931471805599453



def _llw_consts():
    # Block-ones reduction weights [128, 4, 16]: for group g, column
    # j = 4g+bp sums partition rows 32bp..32bp+31 with weight -1.  The rhs
    # is E = 0.5*u^2 + log_scale, so one matmul yields the full reduction.
    llw = np.zeros((128, 4, 16), np.float32)
    for g in range(4):
        for bp in range(4):
            llw[32 * bp : 32 * (bp + 1), g, 4 * g + bp] = -1.0
    return llw


def build_nc(n_total=N):
    assert n_total % CHUNK == 0
    n_chunks = n_total // CHUNK

    nc = bacc.Bacc(
        "TRN2",
        target_bir_lowering=False,
        debug=False,
        enable_asserts=False,
        num_devices=N_CORES,
    )

    xtb_d = nc.declare_dram_parameter("xtb", [128, n_total], BF16, isOutput=False)
    wm1_d = nc.declare_dram_parameter("wm1", [128, 2, 4, 128], BF16, isOutput=False)
    wm2_d = nc.declare_dram_parameter("wm2", [128, 2, 16, 128], BF16, isOutput=False)
    wm3_d = nc.declare_dram_parameter(
        "wm3", [128, 2, 16, 2, 32], BF16, isOutput=False
    )
    out_d = nc.declare_dram_parameter("out", [n_chunks, 16, CHUNK], F32, isOutput=True)

    llw_d = nc.inline_tensor(_llw_consts(), "llw")

    with TileContext(nc) as tc:
        with (
            tc.tile_pool(name="const", bufs=1) as cpool,
            tc.tile_pool(name="wload", bufs=2) as lpool,
            tc.tile_pool(name="hstage", bufs=2) as hpool,
            tc.tile_pool(name="tail", bufs=3) as tpool,
            tc.tile_pool(name="llout", bufs=2) as opool,
            tc.tile_pool(name="p12", bufs=2, space="PSUM") as p12pool,
            tc.tile_pool(name="pt", bufs=1, space="PSUM") as ptpool,
            tc.tile_pool(name="pl", bufs=2, space="PSUM") as plpool,
            tc.tile_pool(name="pll", bufs=1, space="PSUM") as pllpool,
        ):
            # ---- persistent inputs -------------------------------------
            xtb = cpool.tile([128, n_total], BF16, tag="xtb")
            nc.sync.dma_start(out=xtb[:], in_=xtb_d[:])

            # ---- masked weights (w * m), kept resident ----------------
            # Only what L1(chunk 0) needs is loaded up front; everything else
            # (w2m/w3m groups, llw, exp-bias) is deferred into the early
            # pipeline quanta so the first matmul issues as soon as possible.
            w1m = cpool.tile([128, 4, 128], BF16, tag="w1m")
            w2m = cpool.tile([128, 16, 128], BF16, tag="w2m")
            w3m = cpool.tile([128, 16, 2, 32], BF16, tag="w3m")
            llw = cpool.tile([128, 4, 16], BF16, tag="llw")
            bias_ln = cpool.tile([128, 1], F32, tag="biasln")
            wm1raw = lpool.tile([128, 2, 4, 128], BF16, tag="l1")
            nc.sync.dma_start(out=wm1raw[:], in_=wm1_d[:])
            nc.vector.tensor_mul(out=w1m[:], in0=wm1raw[:, 0], in1=wm1raw[:, 1])

            def load_w2(g):
                bs = slice(4 * g, 4 * (g + 1))
                wm2raw = lpool.tile([128, 2, 4, 128], BF16, tag="l2")
                nc.sync.dma_start(out=wm2raw[:], in_=wm2_d[:, :, bs, :])
                nc.vector.tensor_mul(
                    out=w2m[:, bs, :], in0=wm2raw[:, 0], in1=wm2raw[:, 1]
                )

            def load_w3(g):
                bs = slice(4 * g, 4 * (g + 1))
                wm3raw = lpool.tile([128, 2, 4, 2, 32], BF16, tag="l3")
                nc.sync.dma_start(out=wm3raw[:], in_=wm3_d[:, :, bs, :, :])
                nc.vector.tensor_mul(
                    out=w3m[:, bs, :, :], in0=wm3raw[:, 0], in1=wm3raw[:, 1]
                )

            def load_llw():
                llwstage = lpool.tile([128, 4, 16], F32, tag="llwf")
                nc.sync.dma_start(out=llwstage[:], in_=llw_d[:])
                nc.vector.tensor_copy(out=llw[:], in_=llwstage[:])
                nc.gpsimd.memset(bias_ln[:], LN_HALF)

            deferred_loads = [
                lambda: load_w2(0), lambda: load_w2(1),
                lambda: load_w2(2), lambda: load_w2(3),
                lambda: load_w3(0), lambda: load_w3(1),
                lambda: load_w3(2), lambda: load_w3(3),
                load_llw,
            ]

            # ---- pipelined stages --------------------------------------
            # Each pair-relu is split asymmetrically across BOTH engines in
            # parallel (ACT is 1.2 GHz with 172c PSUM init, DVE 0.96 GHz with
            # 120c): the PSUM pair frees after max(623, 592) ns instead of a
            # single ~1.1 us op, shortening the matmul->relu->matmul
            # round-trip that paces the whole pipeline.
            SPLIT = 640

            def relu_split(h_flat, base, pair_ap):
                nc.scalar.activation(
                    h_flat[:, base : base + SPLIT],
                    pair_ap[:, 0:SPLIT],
                    mybir.ActivationFunctionType.Relu,
                )
                nc.vector.tensor_scalar_max(
                    h_flat[:, base + SPLIT : base + 2 * CHUNK],
                    pair_ap[:, SPLIT : 2 * CHUNK],
                    0.0,
                )

            state = {}

            def l1_quantum(c, i):
                # quantum i in 0..7: one pair (2 row-packed K=32 matmuls + relu)
                cs = slice(c * CHUNK, (c + 1) * CHUNK)
                st = state.setdefault(c, {})
                if i == 0:
                    h1a_t = hpool.tile([128, 16 * CHUNK], BF16, tag="h1a")
                    st["h1a"] = h1a_t
                h1a = st["h1a"]
                g, j = divmod(i, 2)
                pair = p12pool.tile([128, 2 * CHUNK], F32, tag="p12")
                for k in range(2):
                    bp = 2 * j + k
                    prow = slice(32 * bp, 32 * (bp + 1))
                    nc.tensor.matmul(
                        pair[:, k * CHUNK : (k + 1) * CHUNK],
                        w1m[prow, g, :],
                        xtb[prow, cs],
                        start=True,
                        stop=True,
                        tile_position=(32 * bp, 0),
                    )
                b0 = 4 * g + 2 * j
                relu_split(h1a, b0 * CHUNK, pair)

            def l2_quantum(c, i):
                st = state[c]
                if i == 0:
                    h2a_t = hpool.tile([128, 16 * CHUNK], BF16, tag="h2a")
                    st["h2a"] = h2a_t
                h1a, h2a = st["h1a"], st["h2a"]
                pair = p12pool.tile([128, 2 * CHUNK], F32, tag="p12")
                for k in range(2):
                    b = 2 * i + k
                    nc.tensor.matmul(
                        pair[:, k * CHUNK : (k + 1) * CHUNK],
                        w2m[:, b, :],
                        h1a[:, b * CHUNK : (b + 1) * CHUNK],
                        start=True,
                        stop=True,
                    )
                relu_split(h2a, 2 * i * CHUNK, pair)

            def x_quantum(c, x):
                # Three-phase software-pipelined tail so every op is
                # dependency-clear when its in-order engine queue reaches it:
                #   PH1(g) at x=g:   8 col-packed L3 matmuls + T = tps - x
                #   PH2(g) at x=g+1: B = exp(-2l + ln0.5), A = T*T, C = A*B
                #   PH3(g) at x=g+2: E = l + C (stt), ll matmul
                # Emission order inside a quantum: oldest phase first.
                cs = slice(c * CHUNK, (c + 1) * CHUNK)
                st = state[c]
                if x == 0:
                    llps_t = pllpool.tile([16, CHUNK], F32, tag="llps")
                    st["llps"] = llps_t
                    st["ph"] = {}
                llps = st["llps"]
                ph = st["ph"]

                g3 = x - 2
                if 0 <= g3 <= 3:
                    lps, c_sb = ph.pop(("ph3", g3))
                    e_sb = tpool.tile([128, CHUNK], BF16, tag="e")
                    nc.vector.scalar_tensor_tensor(
                        out=e_sb[:],
                        in0=lps[:],
                        scalar=1.0,
                        in1=c_sb[:],
                        op0=mybir.AluOpType.mult,
                        op1=mybir.AluOpType.add,
                    )
                    nc.tensor.matmul(
                        llps[:],
                        llw[:, g3, :],
                        e_sb[:],
                        start=(g3 == 0),
                        stop=(g3 == 3),
                        skip_group_check=True,
                    )

                g2 = x - 1
                if 0 <= g2 <= 3:
                    tps, lps, t_sb = ph.pop(("ph2", g2))
                    b_sb = tpool.tile([128, CHUNK], BF16, tag="b")
                    nc.scalar.activation(
                        b_sb[:],
                        lps[:],
                        mybir.ActivationFunctionType.Exp,
                        scale=-2.0,
                        bias=bias_ln[:],
                    )
                    a_sb = tpool.tile([128, CHUNK], BF16, tag="a")
                    nc.gpsimd.tensor_mul(out=a_sb[:], in0=t_sb[:], in1=t_sb[:])
                    c_sb = tpool.tile([128, CHUNK], BF16, tag="c")
                    nc.gpsimd.tensor_mul(out=c_sb[:], in0=a_sb[:], in1=b_sb[:])
                    ph[("ph3", g2)] = (lps, c_sb)

                g = x
                if 0 <= g <= 3:
                    h2a = st["h2a"]
                    tps = ptpool.tile([128, CHUNK], F32, tag="tps")
                    lps = plpool.tile([128, CHUNK], F32, tag="lps")
                    for bp in range(4):
                        b = 4 * g + bp
                        prow = slice(32 * bp, 32 * (bp + 1))
                        nc.tensor.matmul(
                            tps[prow, :],
                            w3m[:, b, 0, :],
                            h2a[:, b * CHUNK : (b + 1) * CHUNK],
                            start=True,
                            stop=True,
                            tile_position=(0, 32 * bp),
                        )
                    for bp in range(4):
                        b = 4 * g + bp
                        prow = slice(32 * bp, 32 * (bp + 1))
                        nc.tensor.matmul(
                            lps[prow, :],
                            w3m[:, b, 1, :],
                            h2a[:, b * CHUNK : (b + 1) * CHUNK],
                            start=True,
                            stop=True,
                            tile_position=(0, 32 * bp),
                        )
                    t_sb = tpool.tile([128, CHUNK], BF16, tag="t")
                    nc.vector.tensor_sub(out=t_sb[:], in0=tps[:], in1=xtb[:, cs])
                    ph[("ph2", g)] = (tps, lps, t_sb)

                if x == 5:
                    ll_sb = opool.tile([16, CHUNK], F32, tag="ll")
                    nc.scalar.activation(
                        ll_sb[:], llps[:], mybir.ActivationFunctionType.Copy
                    )
                    nc.sync.dma_start(out=out_d[c], in_=ll_sb[:])
                    del state[c]

            # Fine-grained round-robin across three pipelined stages (oldest
            # first) so each engine's in-order queue always has ready work.
            X_AT = {0: 0, 2: 1, 3: 2, 5: 3, 6: 4, 7: 5}
            for step in range(n_chunks + 2):
                for i in range(8):
                    if step >= 2 and step - 2 < n_chunks and i in X_AT:
                        x_quantum(step - 2, X_AT[i])
                    if 1 <= step <= n_chunks:
                        l2_quantum(step - 1, i)
                    if step < n_chunks:
                        l1_quantum(step, i)
                    if deferred_loads and step == 0:
                        deferred_loads.pop(0)()
                if deferred_loads and step == 0:
                    while deferred_loads:
                        deferred_loads.pop(0)()

    nc.compile()
    return nc


def shard_inputs(x, W1, W2, W3, M1, M2, M3, region_idx, n_total=N):
    """Per-core input dicts: pure gather/transpose/replicate layout prep."""
    x = np.asarray(x, dtype=np.float32)
    region_idx = np.asarray(region_idx)
    in_maps = []
    for r in range(N_CORES):
        xr = x[:n_total, region_idx[r]]  # [n, D]
        xt = np.ascontiguousarray(xr.T)  # [D, n]
        xt4 = np.ascontiguousarray(np.tile(xt, (4, 1)))  # [128, n]

        def prep1(w):
            w = np.asarray(w[r], dtype=np.float32)  # [16, 32, 128]
            return np.ascontiguousarray(
                w.reshape(4, 4, D, H).transpose(1, 2, 0, 3).reshape(128, 4, H)
            ).astype(ml_dtypes.bfloat16)

        def prep2(w):
            w = np.asarray(w[r], dtype=np.float32)  # [16, 128, 128]
            return np.ascontiguousarray(w.transpose(1, 0, 2)).astype(
                ml_dtypes.bfloat16
            )

        def prep3(w):
            w = np.asarray(w[r], dtype=np.float32)  # [16, 128, 64]
            return np.ascontiguousarray(
                w.reshape(B, H, D, 2).transpose(1, 0, 3, 2)
            ).astype(ml_dtypes.bfloat16)

        in_maps.append(
            {
                "xtb": xt4.astype(ml_dtypes.bfloat16),
                "wm1": np.ascontiguousarray(
                    np.stack([prep1(W1), prep1(M1)], axis=1)
                ),
                "wm2": np.ascontiguousarray(
                    np.stack([prep2(W2), prep2(M2)], axis=1)
                ),
                "wm3": np.ascontiguousarray(
                    np.stack([prep3(W3), prep3(M3)], axis=1)
                ),
            }
        )
    return in_maps


_NC_CACHE = {}


def run(x, W1, W2, W3, M1, M2, M3, region_idx, trace=False, n_total=N):
    if n_total not in _NC_CACHE:
        _NC_CACHE[n_total] = build_nc(n_total)
    nc = _NC_CACHE[n_total]
    in_maps = shard_inputs(x, W1, W2, W3, M1, M2, M3, region_idx, n_total)
    res = run_bass_kernel_spmd(
        nc, in_maps, core_ids=list(range(N_CORES)), trace=trace
    )
    out = np.empty((n_total, R, B), dtype=np.float32)
    for r in range(N_CORES):
        o = res.results[r]["out"]  # [n_chunks, 16, CHUNK]
        out[:, r, :] = o.transpose(0, 2, 1).reshape(n_total, B)
    out += np.float32(-D * HALF_LOG_2PI)
    return out, res


def kernel(x, W1, W2, W3, M1, M2, M3, region_idx):
    out, _ = run(x, W1, W2, W3, M1, M2, M3, region_idx)
    return out
